# revision 4
# baseline (speedup 1.0000x reference)
"""AtomToPair GNN message-passing kernel for 8 TRN2 NeuronCores.

Math (per molecule, A=64 atoms, F=C=128):
    h0[i,j] = MLP([x_i, x_j]),  h1[i,j] = MLP([x_j, x_i]) = h0[j,i]
    out[i,j] = h0[i,j] + h0[j,i]           (symmetric in i,j!)
so a single MLP pass over all A*A pairs suffices, followed by a
transposed add over the pair grid — and since out is symmetric we only
compute/store the block-upper-triangle (j >= 8*floor(i/8)) and mirror
on the host.

Layer 1 factors per atom: [x_i,x_j]@W0 = x_i@W0top + x_j@W0bot, computed
on the TensorEngine as two accumulated bf16 matmuls whose moving operand
reads xT with broadcast/tiled access patterns (no pair tensor is ever
materialized).  Matmuls run in bf16 (fp32 matmul on TRN2 is the slow
LOW_HIGH two-pass mode); PSUM accumulation stays fp32 and the final
output is fp32.

Sharding: data-parallel over batch — each of the 8 cores handles B/8 = 4
molecules with fully replicated weights. On-chip compute is
feature-major ([C on partitions, pairs on free]); the host transposes
to the reference layout during the unshard step.
"""

import sys

sys.path.insert(0, "/opt/trn_rl_repo")

import numpy as np

B, A, F, C = 32, 64, 128, 128
NCORES = 8
MPC = B // NCORES          # molecules per core
PAIRS = A * A              # 4096
IB = 8                     # i-block (rows per chunk)
NCHUNK = A // IB           # 8 chunks per molecule
# packed block-triangle: chunk k holds rows i in [8k,8k+8), cols j in [8k,64)
TRI_W = [A - IB * k for k in range(NCHUNK)]      # 64,56,...,8
TRI_OFF = [IB * sum(TRI_W[:k]) for k in range(NCHUNK)]
TRI_COLS = IB * sum(TRI_W)                        # 2304 per molecule

# packed bf16 param columns: xT | w0t | w0b | w1
XB_OFF, W0T_OFF, W0B_OFF, W1_OFF = 0, MPC * A, MPC * A + C, MPC * A + 2 * C
PB_COLS = MPC * A + 3 * C

N_WARMUP = 24

_compiled = None


def _build():
    import concourse.bass as bass
    import concourse.tile as tile
    from concourse import bacc, mybir

    fp32 = mybir.dt.float32
    bf16 = mybir.dt.bfloat16
    nc = bacc.Bacc("TRN2", target_bir_lowering=False, debug=False,
                   num_devices=NCORES)

    pb16 = nc.dram_tensor("pb16", [128, PB_COLS], bf16,
                          kind="ExternalInput").ap()
    pf32 = nc.dram_tensor("pf32", [128, 2], fp32, kind="ExternalInput").ap()
    out = nc.dram_tensor("out", [C, MPC * TRI_COLS], fp32,
                         kind="ExternalOutput").ap()

    Relu = mybir.ActivationFunctionType.Relu
    add_op = mybir.AluOpType.add
    max_op = mybir.AluOpType.max

    with tile.TileContext(nc) as tc:
        with (
            tc.tile_pool(name="const", bufs=1) as const_pool,
            tc.tile_pool(name="warm", bufs=1) as warm_pool,
            tc.tile_pool(name="y1", bufs=3) as y1_pool,
            tc.tile_pool(name="hrelu", bufs=2) as h_pool,
            tc.tile_pool(name="obuf", bufs=2) as o_pool,
            tc.tile_pool(name="psY", bufs=2, space="PSUM") as psY_pool,
            tc.tile_pool(name="psH", bufs=2, space="PSUM") as psH_pool,
        ):
            # PE warm-up: dense dummy matmuls with no input dependency so
            # the HAM clock-gate reaches 8/8 before the real work arrives.
            wsrc = warm_pool.tile([128, 512], bf16, tag="wsrc")
            nc.gpsimd.memset(wsrc[:], 0.0)
            for w in range(N_WARMUP):
                wp = psY_pool.tile([128, 512], fp32, tag="psy2")
                nc.tensor.matmul(wp[:], wsrc[:, :128], wsrc[:],
                                 start=True, stop=True)

            pb = const_pool.tile([128, PB_COLS], bf16, tag="pb")
            nc.sync.dma_start(pb[:], pb16[:])
            pf = const_pool.tile([128, 2], fp32, tag="pf")
            nc.sync.dma_start(pf[:], pf32[:])
            w0t_s = pb[:, W0T_OFF: W0T_OFF + C]
            w0b_s = pb[:, W0B_OFF: W0B_OFF + C]
            w1_s = pb[:, W1_OFF: W1_OFF + C]
            b0_s = pf[:, 0:1]
            b1_s = pf[:, 1:2]

            for m in range(MPC):
                hr = h_pool.tile([C, PAIRS], fp32, tag="hr")
                xm = pb[:, XB_OFF + m * A: XB_OFF + (m + 1) * A]
                # two chunks (2*IB i-values = 1024 pairs) per pipeline step
                for q in range(NCHUNK // 2):
                    psy = psY_pool.tile([C, 2 * IB * A], fp32, tag="psy2")
                    for h in range(2):
                        k = 2 * q + h
                        xi = xm[:, k * IB: (k + 1) * IB]
                        rhs_i = xi.unsqueeze(2).to_broadcast((F, IB, A))
                        rhs_j = xm.unsqueeze(1).to_broadcast((F, IB, A))
                        ps3 = psy[:, h * IB * A: (h + 1) * IB * A].rearrange(
                            "c (i j) -> c i j", i=IB)
                        nc.tensor.matmul(ps3, w0t_s, rhs_i,
                                         start=True, stop=False)
                        nc.tensor.matmul(ps3, w0b_s, rhs_j,
                                         start=False, stop=True)

                    # relu1 + b0 -> bf16 Y1T   (PSUM -> SBUF)
                    y1t = y1_pool.tile([C, 2 * IB * A], bf16, tag="y1t")
                    if q != 3:
                        nc.scalar.activation(y1t[:], psy[:], Relu,
                                             bias=b0_s)
                    else:
                        nc.vector.tensor_scalar(y1t[:], psy[:], b0_s,
                                                0.0, add_op, max_op)

                    # layer 2
                    psh = psH_pool.tile([C, 2 * IB * A], fp32, tag="psh")
                    nc.tensor.matmul(psh[:, :IB * A], w1_s,
                                     y1t[:, :IB * A], start=True, stop=True)
                    nc.tensor.matmul(psh[:, IB * A:], w1_s,
                                     y1t[:, IB * A:], start=True, stop=True)

                    # relu2 + b1 -> fp32 H    (PSUM -> SBUF)
                    hslice = hr[:, q * 2 * IB * A: (q + 1) * 2 * IB * A]
                    if q != 1:
                        nc.scalar.activation(hslice, psh[:], Relu,
                                             bias=b1_s)
                    else:
                        nc.vector.tensor_scalar(hslice, psh[:], b1_s,
                                                0.0, add_op, max_op)

                # block-triangle mirror add:
                # ot[:, k-block] = H[i,j] + H[j,i],  i in [8k,8k+8), j>=8k
                # even chunks on DVE, odd chunks on GPSIMD (both idle-ish)
                ot = o_pool.tile([C, TRI_COLS], fp32, tag="ot")
                h3 = hr[:].rearrange("c (i j) -> c i j", i=A)
                for k in range(NCHUNK):
                    w = TRI_W[k]
                    straight = h3[:, k * IB: (k + 1) * IB, k * IB:]
                    mirror = h3[:, k * IB:, k * IB: (k + 1) * IB]
                    mirror = mirror.transpose([0, 2, 1])
                    o3 = ot[:, TRI_OFF[k]: TRI_OFF[k] + IB * w].rearrange(
                        "c (i j) -> c i j", i=IB)
                    eng = nc.vector if k % 2 == 0 else nc.gpsimd
                    eng.tensor_tensor(o3, straight, mirror, add_op)
                nc.sync.dma_start(
                    out[:, m * TRI_COLS: (m + 1) * TRI_COLS], ot[:])
    nc.compile()
    return nc


def _get_compiled():
    global _compiled
    if _compiled is None:
        _compiled = _build()
    return _compiled


def _shard_inputs(x, W0, b0, W1, b1):
    import ml_dtypes

    bf = ml_dtypes.bfloat16
    pf32 = np.stack([b0, b1], axis=1).astype(np.float32)  # [128, 2]
    w_cols = np.concatenate(
        [W0[:F], W0[F:], W1], axis=1).astype(bf)          # [128, 3C]
    in_maps = []
    for c in range(NCORES):
        xs = x[c * MPC: (c + 1) * MPC]                    # [MPC, A, F]
        xTs = xs.transpose(2, 0, 1).reshape(F, MPC * A)
        pb16 = np.ascontiguousarray(
            np.concatenate([xTs.astype(bf), w_cols], axis=1))
        in_maps.append({"pb16": pb16, "pf32": pf32})
    return in_maps


def _unshard(results):
    """[C, MPC*TRI_COLS] per core -> full (B, A*A, C) with mirror fill."""
    full = np.empty((B, A, A, C), dtype=np.float32)
    for c in range(NCORES):
        o = results[c]["out"]                     # [C, MPC*TRI_COLS]
        for m in range(MPC):
            bidx = c * MPC + m
            pk = o[:, m * TRI_COLS: (m + 1) * TRI_COLS]
            for k in range(NCHUNK):
                w = TRI_W[k]
                blk = pk[:, TRI_OFF[k]: TRI_OFF[k] + IB * w]
                blk = blk.reshape(C, IB, w).transpose(1, 2, 0)
                full[bidx, k * IB: (k + 1) * IB, k * IB:] = blk
                if k > 0:
                    # mirror: cols j < 8k come from the computed (j,i)
                    full[bidx, k * IB: (k + 1) * IB, : k * IB] = \
                        full[bidx, : k * IB, k * IB: (k + 1) * IB] \
                        .transpose(1, 0, 2)
    return full.reshape(B, A * A, C)


def kernel(x, W0, b0, W1, b1):
    from concourse.bass_utils import run_bass_kernel_spmd

    x = np.asarray(x, dtype=np.float32)
    W0 = np.asarray(W0, dtype=np.float32)
    b0 = np.asarray(b0, dtype=np.float32)
    W1 = np.asarray(W1, dtype=np.float32)
    b1 = np.asarray(b1, dtype=np.float32)

    in_maps = _shard_inputs(x, W0, b0, W1, b1)
    nc = _get_compiled()
    res = run_bass_kernel_spmd(nc, in_maps, core_ids=list(range(NCORES)))
    return _unshard(res.results)


# revision 5
# speedup vs baseline: 1.2334x; 1.2334x over previous
"""AtomToPair GNN message-passing kernel for 8 TRN2 NeuronCores.

Math (per molecule, A=64 atoms, F=C=128):
    h0[i,j] = MLP([x_i, x_j]),  h1[i,j] = MLP([x_j, x_i]) = h0[j,i]
    out[i,j] = h0[i,j] + h0[j,i]           (symmetric in i,j!)
so a single MLP pass over all A*A pairs suffices, followed by a
transposed add over the pair grid — and since out is symmetric we only
compute/store the block-upper-triangle (j >= 8*floor(i/8)) and mirror
on the host.

Layer 1 factors per atom: [x_i,x_j]@W0 = x_i@W0top + x_j@W0bot, computed
on the TensorEngine as two accumulated bf16 matmuls whose moving operand
reads xT with broadcast/tiled access patterns (no pair tensor is ever
materialized).  Matmuls run in bf16 (fp32 matmul on TRN2 is the slow
LOW_HIGH two-pass mode); PSUM accumulation stays fp32 and the final
output is fp32.

Sharding: data-parallel over batch — each of the 8 cores handles B/8 = 4
molecules with fully replicated weights. On-chip compute is
feature-major ([C on partitions, pairs on free]); the host transposes
to the reference layout during the unshard step.
"""

import sys

sys.path.insert(0, "/opt/trn_rl_repo")

import numpy as np

B, A, F, C = 32, 64, 128, 128
NCORES = 8
MPC = B // NCORES          # molecules per core
PAIRS = A * A              # 4096
IB = 8                     # i-block (rows per chunk)
NCHUNK = A // IB           # 8 chunks per molecule
# packed block-triangle: chunk k holds rows i in [8k,8k+8), cols j in [8k,64)
TRI_W = [A - IB * k for k in range(NCHUNK)]      # 64,56,...,8
TRI_OFF = [IB * sum(TRI_W[:k]) for k in range(NCHUNK)]
TRI_COLS = IB * sum(TRI_W)                        # 2304 per molecule

# packed bf16 param columns: xT | w0t | w0b | w1
XB_OFF, W0T_OFF, W0B_OFF, W1_OFF = 0, MPC * A, MPC * A + C, MPC * A + 2 * C
PB_COLS = MPC * A + 3 * C

N_WARMUP = 12

_compiled = None


def _build():
    import concourse.bass as bass
    import concourse.tile as tile
    from concourse import bacc, mybir

    fp32 = mybir.dt.float32
    bf16 = mybir.dt.bfloat16
    nc = bacc.Bacc("TRN2", target_bir_lowering=False, debug=False,
                   num_devices=NCORES)

    pb16 = nc.dram_tensor("pb16", [128, PB_COLS], bf16,
                          kind="ExternalInput").ap()
    pf32 = nc.dram_tensor("pf32", [128, 2], fp32, kind="ExternalInput").ap()
    out = nc.dram_tensor("out", [C, MPC * TRI_COLS], fp32,
                         kind="ExternalOutput").ap()

    Relu = mybir.ActivationFunctionType.Relu
    add_op = mybir.AluOpType.add
    max_op = mybir.AluOpType.max

    with tile.TileContext(nc) as tc:
        with (
            tc.tile_pool(name="const", bufs=1) as const_pool,
            tc.tile_pool(name="warm", bufs=1) as warm_pool,
            tc.tile_pool(name="y1", bufs=3) as y1_pool,
            tc.tile_pool(name="hrelu", bufs=2) as h_pool,
            tc.tile_pool(name="obuf", bufs=2) as o_pool,
            tc.tile_pool(name="psY", bufs=2, space="PSUM") as psY_pool,
            tc.tile_pool(name="psH", bufs=2, space="PSUM") as psH_pool,
        ):
            # PE warm-up: dense dummy matmuls with no input dependency so
            # the HAM clock-gate reaches 8/8 before the real work arrives.
            wsrc = warm_pool.tile([128, 512], bf16, tag="wsrc")
            nc.gpsimd.memset(wsrc[:], 0.0)
            for w in range(N_WARMUP):
                wp = psY_pool.tile([128, 512], fp32, tag="psy2")
                nc.tensor.matmul(wp[:], wsrc[:, :128], wsrc[:],
                                 start=True, stop=True)

            pb = const_pool.tile([128, PB_COLS], bf16, tag="pb")
            nc.sync.dma_start(pb[:], pb16[:])
            pf = const_pool.tile([128, 2], fp32, tag="pf")
            nc.sync.dma_start(pf[:], pf32[:])
            w0t_s = pb[:, W0T_OFF: W0T_OFF + C]
            w0b_s = pb[:, W0B_OFF: W0B_OFF + C]
            w1_s = pb[:, W1_OFF: W1_OFF + C]
            b0_s = pf[:, 0:1]
            b1_s = pf[:, 1:2]

            for m in range(MPC):
                hr = h_pool.tile([C, PAIRS], fp32, tag="hr")
                ot = o_pool.tile([C, TRI_COLS], fp32, tag="ot")
                h3 = hr[:].rearrange("c (i j) -> c i j", i=A)

                def emit_E(k):
                    # ot[:, k-block] = H[i,j] + H[j,i], i in [8k,8k+8), j>=8k
                    w = TRI_W[k]
                    straight = h3[:, k * IB: (k + 1) * IB, k * IB:]
                    mirror = h3[:, k * IB:, k * IB: (k + 1) * IB]
                    mirror = mirror.transpose([0, 2, 1])
                    o3 = ot[:, TRI_OFF[k]: TRI_OFF[k] + IB * w].rearrange(
                        "c (i j) -> c i j", i=IB)
                    nc.vector.tensor_tensor(o3, straight, mirror, add_op)

                xm = pb[:, XB_OFF + m * A: XB_OFF + (m + 1) * A]
                # two chunks (2*IB i-values = 1024 pairs) per pipeline
                # step; REVERSED order so E blocks (needing chunks >= k)
                # become ready progressively during the molecule
                for q in reversed(range(NCHUNK // 2)):
                    psy = psY_pool.tile([C, 2 * IB * A], fp32, tag="psy2")
                    for h in range(2):
                        k = 2 * q + h
                        xi = xm[:, k * IB: (k + 1) * IB]
                        rhs_i = xi.unsqueeze(2).to_broadcast((F, IB, A))
                        rhs_j = xm.unsqueeze(1).to_broadcast((F, IB, A))
                        ps3 = psy[:, h * IB * A: (h + 1) * IB * A].rearrange(
                            "c (i j) -> c i j", i=IB)
                        nc.tensor.matmul(ps3, w0t_s, rhs_i,
                                         start=True, stop=False)
                        nc.tensor.matmul(ps3, w0b_s, rhs_j,
                                         start=False, stop=True)

                    # relu1 + b0 -> bf16 Y1T   (PSUM -> SBUF)
                    y1t = y1_pool.tile([C, 2 * IB * A], bf16, tag="y1t")
                    if q != 0:
                        nc.scalar.activation(y1t[:], psy[:], Relu,
                                             bias=b0_s)
                    else:
                        nc.vector.tensor_scalar(y1t[:], psy[:], b0_s,
                                                0.0, add_op, max_op)

                    # layer 2
                    psh = psH_pool.tile([C, 2 * IB * A], fp32, tag="psh")
                    nc.tensor.matmul(psh[:, :IB * A], w1_s,
                                     y1t[:, :IB * A], start=True, stop=True)
                    nc.tensor.matmul(psh[:, IB * A:], w1_s,
                                     y1t[:, IB * A:], start=True, stop=True)

                    # relu2 + b1 -> fp32 H    (PSUM -> SBUF)
                    hslice = hr[:, q * 2 * IB * A: (q + 1) * 2 * IB * A]
                    if q != 2:
                        nc.scalar.activation(hslice, psh[:], Relu,
                                             bias=b1_s)
                    else:
                        nc.vector.tensor_scalar(hslice, psh[:], b1_s,
                                                0.0, add_op, max_op)

                    emit_E(2 * q)
                    emit_E(2 * q + 1)

                # two output DMAs: tail region (blocks 4..7, ready first)
                # then blocks 0..3
                ob = out[:, m * TRI_COLS: (m + 1) * TRI_COLS]
                nc.sync.dma_start(ob[:, TRI_OFF[4]:], ot[:, TRI_OFF[4]:])
                nc.sync.dma_start(ob[:, :TRI_OFF[4]], ot[:, :TRI_OFF[4]])
    nc.compile()
    return nc


def _get_compiled():
    global _compiled
    if _compiled is None:
        _compiled = _build()
    return _compiled


def _shard_inputs(x, W0, b0, W1, b1):
    import ml_dtypes

    bf = ml_dtypes.bfloat16
    pf32 = np.stack([b0, b1], axis=1).astype(np.float32)  # [128, 2]
    w_cols = np.concatenate(
        [W0[:F], W0[F:], W1], axis=1).astype(bf)          # [128, 3C]
    in_maps = []
    for c in range(NCORES):
        xs = x[c * MPC: (c + 1) * MPC]                    # [MPC, A, F]
        xTs = xs.transpose(2, 0, 1).reshape(F, MPC * A)
        pb16 = np.ascontiguousarray(
            np.concatenate([xTs.astype(bf), w_cols], axis=1))
        in_maps.append({"pb16": pb16, "pf32": pf32})
    return in_maps


def _unshard(results):
    """[C, MPC*TRI_COLS] per core -> full (B, A*A, C) with mirror fill."""
    full = np.empty((B, A, A, C), dtype=np.float32)
    for c in range(NCORES):
        o = results[c]["out"]                     # [C, MPC*TRI_COLS]
        for m in range(MPC):
            bidx = c * MPC + m
            pk = o[:, m * TRI_COLS: (m + 1) * TRI_COLS]
            for k in range(NCHUNK):
                w = TRI_W[k]
                blk = pk[:, TRI_OFF[k]: TRI_OFF[k] + IB * w]
                blk = blk.reshape(C, IB, w).transpose(1, 2, 0)
                full[bidx, k * IB: (k + 1) * IB, k * IB:] = blk
                if k > 0:
                    # mirror: cols j < 8k come from the computed (j,i)
                    full[bidx, k * IB: (k + 1) * IB, : k * IB] = \
                        full[bidx, : k * IB, k * IB: (k + 1) * IB] \
                        .transpose(1, 0, 2)
    return full.reshape(B, A * A, C)


def kernel(x, W0, b0, W1, b1):
    from concourse.bass_utils import run_bass_kernel_spmd

    x = np.asarray(x, dtype=np.float32)
    W0 = np.asarray(W0, dtype=np.float32)
    b0 = np.asarray(b0, dtype=np.float32)
    W1 = np.asarray(W1, dtype=np.float32)
    b1 = np.asarray(b1, dtype=np.float32)

    in_maps = _shard_inputs(x, W0, b0, W1, b1)
    nc = _get_compiled()
    res = run_bass_kernel_spmd(nc, in_maps, core_ids=list(range(NCORES)))
    return _unshard(res.results)
